# revision 3
# baseline (speedup 1.0000x reference)
"""Trainium2 Bass kernel: masked multi-head self-attention block.

out = softmax_mask((x @ Wq) (x @ Wk)^T / sqrt(d)) (x @ Wv) @ Wp + b

Sharding: data-parallel over batch B=8 across the 8 NeuronCores (one
batch row per core); weights replicated; no collectives.

Key compaction: masked keys contribute exactly zero, so each core
keeps only the valid key rows of x (padded to a 128 multiple; padded
slots get a -1e30 score bias -> exp = 0). K/V and attention run on
NK ~= 1152 keys instead of 2048.

v6 structure:
  - All layout work on the host: x arrives pre-transposed and
    pre-compacted in bf16, weights arrive bf16 and pre-shuffled into
    SBUF-layout [128, ...] panels so each input is ONE large
    contiguous DMA (few descriptors, full bandwidth).
  - Prelude (PE): K^T[0] -> V (all chunks) -> Q^T[0] first block;
    attention starts ~20us in. Q^T[0] rest, K^T[1..5], Q^T[1..5] are
    computed *during* the attention stream from a backlog using the
    proj PSUM slot.
  - Attention: one flat software-pipelined stream over (qblock,
    headpair, keychunk) steps with S-lookahead 2; merged exp
    [128,2,512] on ScalarE (j==0 writes the running-sum tile
    directly); running-sum adds on DVE; col-packed PV pair;
    denominator matmuls after the next S pair; reciprocal after a
    DRAM broadcast round-trip. proj drains from the same backlog with
    an adaptive drain rate; the last qblock's proj alternates two
    PSUM tags so the tail pipelines.
"""
import numpy as np
import ml_dtypes

import concourse.bass as bass
import concourse.tile as tile
from concourse import bacc, mybir
from concourse.bass_utils import run_bass_kernel_spmd

F32 = mybir.dt.float32
BF16 = mybir.dt.bfloat16

B, N, DIM = 8, 2048, 768
H, D = 12, 64
SCALE = D ** -0.5
NCH = N // 128        # 16 token chunks
KCH = DIM // 128      # 6 feature chunks
QH = 4                # query blocks
QW = N // QH          # 512 queries per block
Exp = mybir.ActivationFunctionType.Exp
BF16_NP = ml_dtypes.bfloat16


def _nslices(w):
    out = [512] * (w // 512)
    if w % 512:
        out.append(w % 512)
    return out


def _build(nc, tc, aps, nkc):
    xT_d, xgT_d, wkv_d, wq_d, wp_d, cst_d, o_d = aps
    NK = nkc * 128

    cpool = tc.alloc_tile_pool(name="const", bufs=1)
    ones_c = cpool.tile([128, 128], BF16)
    nc.vector.memset(ones_c, 0.0)
    nc.vector.memset(ones_c[:, 0:1], 1.0)
    cst_t = cpool.tile([128, nkc + DIM], F32)
    nc.sync.dma_start(out=cst_t, in_=cst_d)
    kb_t = cst_t[:, 0:nkc]
    bp_bc = cst_t[:, nkc:nkc + DIM]

    # persistent tiles (live through attention; released at the end)
    qkvpool = tc.alloc_tile_pool(name="qkv_sb", bufs=1)
    qt = [qkvpool.tile([128, N], BF16, tag=f"qt{m}", name=f"qt{m}")
          for m in range(KCH)]
    kt = [qkvpool.tile([128, NK], BF16, tag=f"kt{m}", name=f"kt{m}")
          for m in range(KCH)]
    v_nat = [qkvpool.tile([128, DIM], BF16, tag=f"vn{t}", name=f"vn{t}")
             for t in range(nkc)]
    ot = [[qkvpool.tile([128, QW], BF16, tag=f"ot{c}_{q}", name=f"ot{c}_{q}")
           for q in range(QH)] for c in range(KCH)]
    # big packed panels: per-chunk data at fixed column offsets
    xct_all = qkvpool.tile([128, KCH * NK], BF16, tag="xct", name="xct")
    wkv_sb = qkvpool.tile([128, KCH * 1536], BF16, tag="wkv", name="wkv")
    xt_all = qkvpool.tile([128, KCH * N], BF16, tag="xt", name="xt")
    wq_sb = qkvpool.tile([128, KCH * DIM], BF16, tag="wqq", name="wqq")
    wp_sb = qkvpool.tile([128, KCH * DIM], BF16, tag="wpp", name="wpp")

    def xct(c):
        return xct_all[:, c * NK:(c + 1) * NK]

    def xt(c):
        return xt_all[:, c * N:(c + 1) * N]

    def wk(c, m):
        o = c * 1536 + m * 128
        return wkv_sb[:, o:o + 128]

    def wv(c, lo, hi):
        o = c * 1536 + 768
        return wkv_sb[:, o + lo:o + hi]

    def wqc(c, m):
        o = c * DIM + m * 128
        return wq_sb[:, o:o + 128]

    def wp(c, lo, hi):
        o = c * DIM
        return wp_sb[:, o + lo:o + hi]

    # DMA loads, ordered by when compute needs them; each input is one
    # or two big contiguous transfers, split across issuing queues.
    half = (KCH // 2) * NK
    nc.gpsimd.dma_start(out=xct_all[:, 0:half], in_=xgT_d[:, 0:half])
    nc.gpsimd.dma_start(out=xct_all[:, half:], in_=xgT_d[:, half:])
    wh = (KCH // 2) * 1536
    nc.scalar.dma_start(out=wkv_sb[:, 0:wh], in_=wkv_d[:, 0:wh])
    nc.scalar.dma_start(out=wkv_sb[:, wh:], in_=wkv_d[:, wh:])
    nc.sync.dma_start(out=xt_all[:, 0:N * (KCH // 2)],
                      in_=xT_d[:, 0:N * (KCH // 2)])
    nc.sync.dma_start(out=xt_all[:, N * (KCH // 2):],
                      in_=xT_d[:, N * (KCH // 2):])
    nc.scalar.dma_start(out=wq_sb, in_=wq_d)
    nc.scalar.dma_start(out=wp_sb, in_=wp_d)

    # K^T chunk 0 (needed by the first attention steps)
    with tc.tile_pool(name="ps_k", bufs=1, space="PSUM") as ps_k:
        mm_ps = ps_k.tile([128, NK], F32, tag="k_ps", name="k_ps")
        for c in range(KCH):
            off = 0
            for w in _nslices(NK):
                nc.tensor.matmul(
                    mm_ps[:, off:off + w],
                    wk(c, 0),
                    xct(c)[:, off:off + w],
                    start=(c == 0), stop=(c == KCH - 1))
                off += w
        nc.scalar.copy(kt[0], mm_ps)
    # V (all chunks; the first pass sweeps all of them in 9 steps)
    with tc.tile_pool(name="ps_v", bufs=2, space="PSUM") as ps_v:
        for t in range(nkc):
            v_ps = ps_v.tile([128, 2, 512], F32, tag="v_ps",
                             name="v_ps")
            for c in range(KCH):
                nc.tensor.matmul(
                    v_ps[:, 0, :],
                    xct(c)[:, t * 128:(t + 1) * 128],
                    wv(c, 0, 512),
                    start=(c == 0), stop=(c == KCH - 1))
                nc.tensor.matmul(
                    v_ps[:, 1, 0:256],
                    xct(c)[:, t * 128:(t + 1) * 128],
                    wv(c, 512, 768),
                    start=(c == 0), stop=(c == KCH - 1))
            nc.vector.tensor_copy(v_nat[t][:, 0:512], v_ps[:, 0, :])
            nc.vector.tensor_copy(v_nat[t][:, 512:DIM],
                                  v_ps[:, 1, 0:256])
    # Q^T chunk 0, first query block only (rest is deferred)
    with tc.tile_pool(name="ps_q0", bufs=1, space="PSUM") as ps_q0:
        mm_ps = ps_q0.tile([128, QW], F32, tag="q_ps", name="q_ps")
        for c in range(KCH):
            nc.tensor.matmul(
                mm_ps, wqc(c, 0), xt(c)[:, 0:QW],
                start=(c == 0), stop=(c == KCH - 1))
        nc.scalar.copy(qt[0][:, 0:QW], mm_ps)

    # deferred Q^T[0] rest + K^T/Q^T chunk closures (run inside the
    # attention stream, borrowing the proj PSUM slot)
    def qk_backlog(ps):
        items = []
        # qt[0] columns 512:2048, one 512-block at a time
        for blk in range(1, QH):
            st = {}
            lo = blk * QW

            def q0step(cc, st=st, lo=lo, first=False):
                if first:
                    st["ps"] = ps.tile([128, QW], F32, tag="pr",
                                       bufs=1, name="kq_ps")
                for c in cc:
                    nc.tensor.matmul(
                        st["ps"], wqc(c, 0), xt(c)[:, lo:lo + QW],
                        start=(c == 0), stop=(c == KCH - 1))

            def fin_q0(st=st, lo=lo):
                nc.vector.tensor_copy(qt[0][:, lo:lo + QW], st["ps"])

            items.append(lambda f=q0step: f((0, 1), first=True))
            items.append(lambda f=q0step: f((2, 3)))
            items.append(lambda f=q0step: f((4, 5)))
            items.append(fin_q0)
        for m in range(1, KCH):
            for lo, hi in [(0, 512), (512, NK)]:
                st = {}
                nsl = _nslices(hi - lo)

                def kstep(c, st=st, m=m, lo=lo, hi=hi, first=False):
                    if first:
                        st["ps"] = ps.tile([128, hi - lo], F32, tag="pr",
                                           bufs=1, name="kq_ps")
                    off = 0
                    for w in _nslices(hi - lo):
                        nc.tensor.matmul(
                            st["ps"][:, off:off + w],
                            wk(c, m),
                            xct(c)[:, lo + off:lo + off + w],
                            start=(c == 0), stop=(c == KCH - 1))
                        off += w

                def fin_k(st=st, m=m, lo=lo, hi=hi):
                    nc.vector.tensor_copy(kt[m][:, lo:hi], st["ps"])

                if len(nsl) == 1:
                    # 1 matmul per c step: pair them up
                    items.append(lambda f=kstep: (f(0, first=True), f(1)))
                    items.append(lambda f=kstep: (f(2), f(3)))
                    items.append(lambda f=kstep: (f(4), f(5)))
                else:
                    for c in range(KCH):
                        items.append(
                            lambda c=c, f=kstep: f(c, first=(c == 0)))
                items.append(fin_k)
            for half in range(2):
                st = {}
                lo = half * 1024

                def qstep(c, st=st, m=m, lo=lo, first=False):
                    if first:
                        st["ps"] = ps.tile([128, 1024], F32, tag="pr",
                                           bufs=1, name="kq_ps")
                    for g in range(2):
                        nc.tensor.matmul(
                            st["ps"][:, g * 512:(g + 1) * 512],
                            wqc(c, m),
                            xt(c)[:, lo + g * 512:lo + (g + 1) * 512],
                            start=(c == 0), stop=(c == KCH - 1))

                def fin_q(st=st, m=m, lo=lo):
                    nc.vector.tensor_copy(qt[m][:, lo:lo + 1024], st["ps"])

                for c in range(KCH):
                    items.append(lambda c=c, f=qstep: f(c, first=(c == 0)))
                items.append(fin_q)
        return items

    # ---------------- attention + proj --------------------------
    _attention(nc, tc, qt, kt, v_nat, kb_t, ones_c, ot, nkc, wp,
               bp_bc, o_d, qk_backlog)
    qkvpool.release()
    cpool.release()


def _attention(nc, tc, qt, kt, v_nat, kb_t, ones_c, ot, nkc,
               wp, bp_bc, o_d, qk_backlog):
    with tc.tile_pool(name="p_sb", bufs=3) as ppool, \
         tc.tile_pool(name="rs_sb", bufs=2) as rspool, \
         tc.tile_pool(name="ep_sb", bufs=3) as eppool, \
         tc.tile_pool(name="out_sb", bufs=3) as outpool, \
         tc.tile_pool(name="dr_sb", bufs=3, space="DRAM") as drpool, \
         tc.tile_pool(name="ps_c", bufs=1, space="PSUM") as ps:

        backlog = qk_backlog(ps)

        def drain(k):
            for _ in range(min(k, len(backlog))):
                backlog.pop(0)()

        def emit_S(qh, hp, j):
            q0 = qh * QW
            s_t = ps.tile([128, 2, 512], F32, tag="s", bufs=2, name="s_t")
            for a in range(2):
                r0 = a * 64
                nc.tensor.matmul(
                    s_t[:, a, :],
                    kt[hp][r0:r0 + 64, j * 128:(j + 1) * 128],
                    qt[hp][r0:r0 + 64, q0:q0 + QW],
                    start=True, stop=True)
            return s_t

        def queue_proj(qh):
            def make_chunk(t_i, tag):
                st = {}

                def cstep(c, t_i=t_i, st=st, tag=tag):
                    if c == 0:
                        if tag == "s":
                            st["pr"] = ps.tile([128, 2, 512], F32, tag="s",
                                               bufs=2, name="pr")
                        else:
                            st["pr"] = ps.tile([128, 2, 512], F32,
                                               tag="pr", bufs=1, name="pr")
                    tl = (t_i % 4) * 128
                    pr = st["pr"]
                    nc.tensor.matmul(
                        pr[:, 0, :], ot[c][t_i // 4][:, tl:tl + 128],
                        wp(c, 0, 512),
                        start=(c == 0), stop=(c == KCH - 1))
                    nc.tensor.matmul(
                        pr[:, 1, 0:256], ot[c][t_i // 4][:, tl:tl + 128],
                        wp(c, 512, DIM),
                        start=(c == 0), stop=(c == KCH - 1))

                def finish(t_i=t_i, st=st):
                    pr = st["pr"]
                    out_t = outpool.tile([128, DIM], F32, tag="out_t",
                                         name="out_t")
                    nc.vector.tensor_add(out_t[:, 0:512], pr[:, 0, :],
                                         bp_bc[:, 0:512])
                    nc.vector.tensor_add(out_t[:, 512:DIM], pr[:, 1, 0:256],
                                         bp_bc[:, 512:DIM])
                    nc.sync.dma_start(
                        out=o_d[t_i * 128:(t_i + 1) * 128, :], in_=out_t)

                return cstep, finish

            if qh < QH - 1:
                for ti in range(4):
                    cstep, finish = make_chunk(qh * 4 + ti, "pr")
                    for c in range(KCH):
                        backlog.append(lambda c=c, f=cstep: f(c))
                    backlog.append(finish)
            else:
                # c-major across 3 concurrent chunks, then the 4th
                chunks = [make_chunk(qh * 4 + ti,
                                     ("s", "s", "pr")[ti])
                          for ti in range(3)]
                for c in range(KCH):
                    for cstep, _ in chunks:
                        backlog.append(lambda c=c, f=cstep: f(c))
                for _, finish in chunks:
                    backlog.append(finish)
                cstep, finish = make_chunk(qh * 4 + 3, "s")
                for c in range(KCH):
                    backlog.append(lambda c=c, f=cstep: f(c))
                backlog.append(finish)

        pending_dn = []
        pending_ep = []

        def epilogue(qh, hp, rs_t, o_t):
            def part1(qh=qh, hp=hp, rs_t=rs_t, o_t=o_t):
                dn_t = ps.tile([128, 2, 512], F32, tag="s", bufs=2,
                               name="dn_t")
                for a in range(2):
                    nc.tensor.matmul(dn_t[:, a, :], ones_c, rs_t[:, a, :],
                                     start=True, stop=True)
                dn_sb = eppool.tile([1, 2, 512], F32, tag="dn_sb",
                                    name="dn_sb")
                nc.vector.tensor_copy(dn_sb, dn_t[0:1, :, :])
                rc_dram = drpool.tile([1024], F32, tag="rc_dram",
                                      name="rc_dram")
                nc.sync.dma_start(out=rc_dram, in_=dn_sb)
                b_raw = eppool.tile([128, QW], F32, tag="b_raw",
                                    name="b_raw", bufs=3)
                for a in range(2):
                    bc_ap = bass.AP(
                        tensor=rc_dram.tensor,
                        offset=rc_dram.offset + a * 512,
                        ap=[[0, 64], [1, 512]])
                    nc.sync.dma_start(out=b_raw[a * 64:(a + 1) * 64, :],
                                      in_=bc_ap)

                def part2(qh=qh, hp=hp, o_t=o_t, b_raw=b_raw):
                    rc_b = eppool.tile([128, QW], F32, tag="rc_b",
                                       name="rc_b", bufs=3)
                    nc.vector.reciprocal_approx_fast(out=rc_b, in_=b_raw)
                    nc.vector.tensor_mul(ot[hp][qh], o_t, rc_b)

                pending_ep.append(part2)

            pending_dn.append(part1)

        tri = [(qh, hp) for hp in range(H // 2) for qh in (0, 1, 2)]
        rest = [(3, hp) for hp in range(H // 2)]
        steps = [(qh, hp, j) for qh, hp in tri + rest
                 for j in range(nkc)]
        s_pend = {}
        s_pend[0] = emit_S(*steps[0])
        s_pend[1] = emit_S(*steps[1])
        hp_state = {}
        for idx, (qh, hp, j) in enumerate(steps):
            if j == 0:
                o_t = ps.tile([128, QW], F32, tag="o", bufs=2, name="o_t")
                rs_t = rspool.tile([128, 2, 512], BF16, tag="rs",
                                   name="rs_t")
                hp_state[(qh, hp)] = (o_t, rs_t)
            o_t, rs_t = hp_state[(qh, hp)]
            s_t = s_pend.pop(idx)
            if j == 0:
                # first chunk's exp writes the running-sum tile directly
                pt_t = rs_t
            else:
                pt_t = ppool.tile([128, 2, 512], BF16, tag="pt",
                                  name="pt_t")
            nc.scalar.activation(pt_t, s_t, Exp,
                                 bias=kb_t[:, j:j + 1], scale=SCALE)
            if idx + 2 < len(steps):
                s_pend[idx + 2] = emit_S(*steps[idx + 2])
            if j > 0:
                nc.vector.tensor_add(rs_t, rs_t, pt_t)
            for a in range(2):
                h = 2 * hp + a
                nc.tensor.matmul(
                    o_t[a * 64:(a + 1) * 64, :],
                    v_nat[j][:, h * D:(h + 1) * D],
                    pt_t[:, a, :],
                    start=(j == 0), stop=(j == nkc - 1),
                    tile_position=(0, a * 64))
            if j == 1:
                while pending_dn:
                    pending_dn.pop(0)()
            if j == 3:
                while pending_ep:
                    pending_ep.pop(0)()
            if j == nkc - 1:
                epilogue(qh, hp, rs_t, o_t)
                del hp_state[(qh, hp)]
                if hp == H // 2 - 1:
                    queue_proj(qh)
            if j == 1:
                pass  # dn matmuls already loaded this step
            elif j == 0 or len(backlog) <= len(steps) - idx:
                drain(1)
            else:
                drain(2)
        while pending_dn:
            pending_dn.pop(0)()
        while pending_ep:
            pending_ep.pop(0)()
        drain(len(backlog))


_CACHE = {}


def _get_compiled(nkc):
    if nkc in _CACHE:
        return _CACHE[nkc]
    NK = nkc * 128
    nc = bacc.Bacc("TRN2", target_bir_lowering=False, debug=False,
                   num_devices=B)
    xT_d = nc.dram_tensor("xT", [128, KCH * N], BF16,
                          kind="ExternalInput").ap()
    xgT_d = nc.dram_tensor("xgT", [128, KCH * NK], BF16,
                           kind="ExternalInput").ap()
    wkv_d = nc.dram_tensor("w_kv", [128, KCH * 1536], BF16,
                           kind="ExternalInput").ap()
    wq_d = nc.dram_tensor("w_q", [128, KCH * DIM], BF16,
                          kind="ExternalInput").ap()
    wp_d = nc.dram_tensor("w_p", [128, KCH * DIM], BF16,
                          kind="ExternalInput").ap()
    cst_d = nc.dram_tensor("cst", [128, nkc + DIM], F32,
                           kind="ExternalInput").ap()
    o_d = nc.dram_tensor("out", [N, DIM], F32, kind="ExternalOutput").ap()
    with tile.TileContext(nc) as tc:
        _build(nc, tc, (xT_d, xgT_d, wkv_d, wq_d, wp_d, cst_d, o_d), nkc)
    nc.compile()
    _CACHE[nkc] = nc
    return nc


def _pack_panel(a):
    """[KCH*128, W] -> [128, KCH*W]: row-chunk c lands at cols c*W."""
    w = a.shape[1]
    return np.ascontiguousarray(
        a.reshape(KCH, 128, w).transpose(1, 0, 2).reshape(128, KCH * w))


def prep_run(x, mask, w_qkv, w_proj, b_proj):
    """Build the compiled program + per-core input maps."""
    x = np.asarray(x, dtype=np.float32)
    mask = np.asarray(mask, dtype=np.int32)

    idxs = [np.flatnonzero(mask[b]).astype(np.int32) for b in range(B)]
    max_valid = max(len(i) for i in idxs)
    nkc = min(NCH, max(1, -(-max_valid // 128)))
    NK = nkc * 128

    xbf = x.astype(BF16_NP)
    wq_np = np.asarray(w_qkv, dtype=np.float32).astype(BF16_NP)
    wkv_p = _pack_panel(wq_np[:, DIM:3 * DIM])
    wq_p = _pack_panel(wq_np[:, 0:DIM])
    wp_p = _pack_panel(np.asarray(w_proj, dtype=np.float32).astype(BF16_NP))
    bp_t = np.tile(np.asarray(b_proj, dtype=np.float32)[None, :], (128, 1))

    in_maps = []
    for b in range(B):
        n = len(idxs[b])
        kidx = np.zeros(NK, dtype=np.int32)
        kidx[:n] = idxs[b]
        kbias = np.full(NK, -1.0e30, dtype=np.float32)
        kbias[:n] = 0.0
        xT = _pack_panel(np.ascontiguousarray(xbf[b].T))
        xgT = _pack_panel(np.ascontiguousarray(xbf[b][kidx].T))
        cst = np.ascontiguousarray(
            np.concatenate([kbias.reshape(nkc, 128).T, bp_t], axis=1))
        in_maps.append({"xT": xT, "xgT": xgT, "w_kv": wkv_p,
                        "w_q": wq_p, "w_p": wp_p, "cst": cst})

    nc = _get_compiled(nkc)
    return nc, in_maps


def kernel(x, mask, w_qkv, w_proj, b_proj):
    nc, in_maps = prep_run(x, mask, w_qkv, w_proj, b_proj)
    last_err = None
    for _ in range(3):
        try:
            res = run_bass_kernel_spmd(nc, in_maps, list(range(B))).results
            return np.stack([res[b]["out"] for b in range(B)], axis=0)
        except Exception as e:  # transient device hiccup: retry
            last_err = e
    raise last_err
